# revision 1
# baseline (speedup 1.0000x reference)
"""Data-dependent ALiBi bias kernel for Trainium2, distributed over 8 NeuronCores.

Reference computation (per full input):
    logits = einsum('bnd,hd->bhn', x, W) + b          # [2, 16, 2048]
    fg     = log_sigmoid(logits)                      # [2, 16, 2048]
    fg     = cumsum(fg, axis=-1)
    out    = fg[:, :, :, None] - fg[:, :, None, :]    # [2, 16, 2048, 2048]

Sharding: 32 (batch, head) pairs / 8 cores = 4 heads per core, batch-major
(cores 0-3 take batch 0, cores 4-7 take batch 1). Each core computes its own
[4, 2048, 2048] slab independently; no collectives.

Device algorithm per core:
    1. logits^T [4, n] via PE matmul of host-pre-transposed x^T (fp16) with
       W^T (fp16), fp32 PSUM accumulate; c-outer / j-inner so matmuls
       pipeline with the x^T chunk DMAs. fp16 inputs halve the input stream
       and run single-pass on the PE (fp32 is double-pumped); end-to-end
       Frobenius rel err 1.9e-5 vs the f32 reference (2.3e-6 all-f32).
    2. u = ln(1 + exp(-(logits + b)))   (= -log_sigmoid(logits), via ACT)
    3. g = cumsum(u)                    (DVE tensor_tensor_scan; g = -fg_cum)
    4. out[h, i, j] = fg_cum[i] - fg_cum[j] = g[j] - g[i]:
       g rows replicated across all 128 partitions by gpsimd
       partition_broadcast (j-term); PE-transposed negated g columns give
       the per-partition i-term bias; one ACT Identity(bias) per
       [128, 2048] tile, then a 1 MB contiguous DMA to DRAM.

Output streaming is the roofline: 64 MB/core at the ~435 GB/s SBUF-AXI DMA
ceiling (~425 GB/s sustained measured). ScalarE generates tiles at
~2.0 us/MB; DMA drains at ~2.4 us/MB; ~205 us/core total on uncontended
cores (~50 us lead-in + ~152 us stream).

Hardware gotchas baked into this design:
  - keep ACT Copy out of the ScalarE stream: mixing ACTIVATE(Copy) with
    Exp/Ln + Identity(bias) hit NRT_EXEC_UNIT_UNRECOVERABLE on hardware
    (table thrash); PSUM->SBUF copies must go to the vector engine.
  - PE matmul/transpose and partition_broadcast operands must sit at base
    partition 0 (or 32/64).
  - one HW wait slot per instruction: more input DMAs than queue
    semaphores gets waits consolidated into "wait for the last DMA".
"""

import numpy as np

B = 2
NH = 16
N = 2048
D = 1024
NCORES = 8
HPC = (B * NH) // NCORES  # 4 (batch, head) pairs per core
P = 128
DC = D // P    # 8 contraction chunks
NCH = N // P   # 16 row chunks per head
NMM = 512      # matmul moving free dim
NJ = N // NMM  # 4

_CACHE = {}


def _build_nc():
    import concourse.bacc as bacc
    import concourse.mybir as mybir
    from concourse.masks import make_identity
    from concourse.tile import TileContext

    f32 = mybir.dt.float32
    Act = mybir.ActivationFunctionType
    nc = bacc.Bacc(None, target_bir_lowering=False)

    xT = nc.dram_tensor("xT", [D, N], mybir.dt.float16, kind="ExternalInput")
    Wt = nc.dram_tensor("Wt", [D, HPC], mybir.dt.float16, kind="ExternalInput")
    bv = nc.dram_tensor("bv", [HPC, 1], f32, kind="ExternalInput")
    out = nc.dram_tensor("out", [HPC, N, N], f32, kind="ExternalOutput")

    with TileContext(nc) as tc:
        with (
            tc.tile_pool(name="big", bufs=1) as big,
            tc.tile_pool(name="small", bufs=1) as small,
            tc.tile_pool(name="grp", bufs=2) as grp,
            tc.tile_pool(name="outp", bufs=10) as outp,
        ):
            ph1 = tc.tile_pool(name="ph1ps", bufs=1, space="PSUM")
            lps = ph1.__enter__()
            gpscm = tc.tile_pool(name="gps", bufs=2, space="PSUM")
            gps = gpscm.__enter__()
            # ---- inputs -> SBUF. Wt first (so ldweights never waits on it);
            # x^T in 4 chunks — one per queue semaphore, so each matmul's
            # single HW wait slot references exactly one DMA.
            f16 = mybir.dt.float16
            Wt_s = small.tile([P, DC, HPC], f16, tag="Wt")
            nc.sync.dma_start(out=Wt_s, in_=Wt.rearrange("(c p) h -> p c h", p=P))
            xT_s = big.tile([P, DC, N], f16, tag="xT")
            xT_r = xT.rearrange("(c p) n -> p c n", p=P)
            # last chunk kept small so the final matmul group retires right
            # after the input stream ends (per-c matmuls wait on whole DMAs)
            for lo, hi in ((0, 2), (2, 4), (4, 7), (7, 8)):
                nc.sync.dma_start(
                    out=xT_s[:, lo:hi, :], in_=xT_r[:, lo:hi, :]
                )
            b_s = small.tile([HPC, 1], f32, tag="b")
            nc.sync.dma_start(out=b_s, in_=bv[:])
            nb = small.tile([HPC, 1], f32, tag="nb")
            nc.vector.tensor_scalar_mul(nb, b_s, -1.0)

            ident = small.tile([HPC, HPC], f32, tag="ident")
            make_identity(nc, ident)
            zeros = small.tile([HPC, N], f32, tag="zeros")
            nc.gpsimd.memset(zeros, 0.0)

            t_exp = small.tile([HPC, N], f32, tag="t_exp")
            g = small.tile([HPC, N], f32, tag="g")
            ngcol = small.tile([P, NCH * HPC], f32, tag="ngcol")
            bcast = big.tile([P, HPC, N], f32, tag="bcast")

            # ---- logits^T [4, n]; each j-group accumulates over c in PSUM,
            # c-outer so group j can retire as soon as the last chunk lands
            # (moving free dim capped at 512 by the PSUM bank on the output)
            MV = 512
            ps = lps.tile([HPC, N], f32, tag="lps")
            for c in range(DC):
                for j in range(N // MV):
                    nc.tensor.matmul(
                        ps[:, j * MV : (j + 1) * MV],
                        Wt_s[:, c, :],
                        xT_s[:, c, j * MV : (j + 1) * MV],
                        start=(c == 0),
                        stop=(c == DC - 1),
                    )
            # t = exp(-(logits + b)); u = ln(1 + t)  (all groups finish
            # together under the c-outer order, so one big EXP + LN;
            # Softplus would fuse these but is absent from the ACT tables)
            nc.scalar.activation(t_exp, ps, Act.Exp, bias=nb[:, 0:1], scale=-1.0)
            nc.scalar.activation(t_exp, t_exp, Act.Ln, bias=1.0)
            # g = cumsum(u)
            nc.vector.tensor_tensor_scan(
                g, t_exp, zeros, 0.0, mybir.AluOpType.add, mybir.AluOpType.add
            )

            # ---- negated g columns: ngcol[p, c*HPC + h] = -g[h, c*P + p]
            for c in range(NCH):
                gp = gps.tile([P, HPC], f32, tag="gps")
                nc.tensor.transpose(gp, g[:, c * P : (c + 1) * P], ident)
                nc.vector.tensor_scalar_mul(
                    ngcol[:, c * HPC : (c + 1) * HPC], gp, -1.0
                )

            gpscm.__exit__(None, None, None)
            ph1.__exit__(None, None, None)

            # ---- bcast[p, h, j] = g[h, j] via gpsimd partition_broadcast
            # (needs its source at partition 0: head 0 reads g directly,
            # heads 1-3 get their row moved down by a tiny SBUF->SBUF DMA)
            nc.gpsimd.partition_broadcast(bcast[:, 0, :], g[0:1, :])
            for h in range(1, HPC):
                grow = grp.tile([1, N], f32, tag="grow")
                nc.sync.dma_start(out=grow, in_=g[h : h + 1, :])
                nc.gpsimd.partition_broadcast(bcast[:, h, :], grow)

            # ---- out[h, c*P + p, :] = g[:] - g[h, c*P + p]
            # (PSUM cannot be a DMA source, so every tile goes via SBUF)
            for h in range(HPC):
                for c in range(NCH):
                    ot = outp.tile([P, N], f32, tag="ot")
                    col = c * HPC + h
                    nc.scalar.activation(
                        ot,
                        bcast[:, h, :],
                        Act.Identity,
                        bias=ngcol[:, col : col + 1],
                        scale=1.0,
                    )
                    nc.sync.dma_start(out=out[h, c * P : (c + 1) * P, :], in_=ot)

    if not nc.is_finalized():
        nc.finalize()
    return nc


def _get_nc():
    if "nc" not in _CACHE:
        _CACHE["nc"] = _build_nc()
    return _CACHE["nc"]


def _make_in_maps(x, W, b):
    x = np.ascontiguousarray(x, dtype=np.float32)
    W = np.ascontiguousarray(W, dtype=np.float32)
    b = np.ascontiguousarray(b, dtype=np.float32)
    xT_by_batch = [np.ascontiguousarray(x[bi].T.astype(np.float16)) for bi in range(B)]
    in_maps = []
    for k in range(NCORES):
        bi = k // (NCORES // B)
        h0 = (k % (NCORES // B)) * HPC
        in_maps.append(
            {
                "xT": xT_by_batch[bi],
                "Wt": np.ascontiguousarray(W[h0 : h0 + HPC].T.astype(np.float16)),
                "bv": np.ascontiguousarray(b[h0 : h0 + HPC].reshape(HPC, 1)),
            }
        )
    return in_maps


def kernel(x, W, b, _trace=False, _trace_cores=None):
    from concourse.bass_utils import run_bass_kernel_spmd

    nc = _get_nc()
    in_maps = _make_in_maps(x, W, b)
    res = run_bass_kernel_spmd(
        nc, in_maps, core_ids=list(range(NCORES)), trace=_trace,
        trace_cores=_trace_cores,
    )
    _CACHE["last_results"] = res
    full = np.empty((B, NH, N, N), dtype=np.float32)
    for k in range(NCORES):
        bi = k // (NCORES // B)
        h0 = (k % (NCORES // B)) * HPC
        full[bi, h0 : h0 + HPC] = res.results[k]["out"]
    return full



# revision 5
# speedup vs baseline: 1.3912x; 1.3912x over previous
"""Data-dependent ALiBi bias kernel for Trainium2, distributed over 8 NeuronCores.

Reference computation (per full input):
    logits = einsum('bnd,hd->bhn', x, W) + b          # [2, 16, 2048]
    fg     = log_sigmoid(logits)                      # [2, 16, 2048]
    fg     = cumsum(fg, axis=-1)
    out    = fg[:, :, :, None] - fg[:, :, None, :]    # [2, 16, 2048, 2048]

Sharding: 32 (batch, head) pairs / 8 cores = 4 heads per core, batch-major
(cores 0-3 take batch 0, cores 4-7 take batch 1). Each core computes its own
[4, 2048, 2048] slab independently; no collectives.

Device algorithm per core (v2 — fp16 output stream):
    1. logits^T [4, n] via PE matmul of host-pre-transposed x^T (fp16) with
       W^T (fp16), fp32 PSUM accumulate; c-outer / j-inner so matmuls
       pipeline with the x^T chunk DMAs.
    2. u = ln(1 + exp(-(logits + b)))   (= -log_sigmoid(logits), via ACT
       Exp then Ln. An explicit early InstLoadActFuncSet of the combined
       natural_log_exp_and_others table — which also contains Identity —
       runs during the input-DMA wait and suppresses the framework's two
       per-function table loads on the critical path.)
    3. g = cumsum(u)                 (DVE tensor_tensor_scan; g = -fg_cum)
    4. out[h, i, j] = fg_cum[i] - fg_cum[j] = g[j] - g[i]:
       g rows replicated across all 128 partitions by gpsimd
       partition_broadcast (j-term); PE-transposed negated g columns give
       the per-partition i-term bias. Tiles are generated alternately by
       ScalarE (ACT Identity+bias) and VectorE (tensor_scalar_add with a
       per-partition scalar) so neither engine gates the fp16 DMA stream,
       and written as fp16 — host upcasts to fp32 on gather. fp16 rounding
       adds ~3e-4 Frobenius rel err (gate is 2e-2).

Output streaming is the roofline: 32 MB/core fp16 at the ~400 GB/s
sustained SBUF->DRAM DMA rate (~80 us) after a ~40 us lead-in.

Hardware gotchas baked into this design:
  - keep ACT Copy out of the ScalarE stream (table thrash on HW); all
    ScalarE ops here live in the one softplus table set.
  - PE matmul/transpose and partition_broadcast operands must sit at base
    partition 0 (or 32/64).
  - one HW wait slot per instruction: more input DMAs than queue
    semaphores gets waits consolidated into "wait for the last DMA".
"""

import numpy as np

B = 2
NH = 16
N = 2048
D = 1024
NCORES = 8
HPC = (B * NH) // NCORES  # 4 (batch, head) pairs per core
P = 128
DC = D // P    # 8 contraction chunks
NCH = N // P   # 16 row chunks per head

_CACHE = {}


def _build_nc():
    import concourse.bacc as bacc
    import concourse.mybir as mybir
    from concourse.masks import make_identity
    from concourse.tile import TileContext

    f32 = mybir.dt.float32
    f16 = mybir.dt.float16
    Act = mybir.ActivationFunctionType
    nc = bacc.Bacc(None, target_bir_lowering=False)

    xT = nc.dram_tensor("xT", [D, N], f16, kind="ExternalInput")
    Wt = nc.dram_tensor("Wt", [D, HPC], f16, kind="ExternalInput")
    bv = nc.dram_tensor("bv", [HPC, 1], f32, kind="ExternalInput")
    out = nc.dram_tensor("out", [HPC, N, N], f16, kind="ExternalOutput")

    with TileContext(nc) as tc:
        with (
            tc.tile_pool(name="big", bufs=1) as big,
            tc.tile_pool(name="small", bufs=1) as small,
            tc.tile_pool(name="grp", bufs=2) as grp,
            tc.tile_pool(name="outp", bufs=12) as outp,
        ):
            ph1 = tc.tile_pool(name="ph1ps", bufs=1, space="PSUM")
            lps = ph1.__enter__()
            gpscm = tc.tile_pool(name="gps", bufs=2, space="PSUM")
            gps = gpscm.__enter__()
            # ---- inputs -> SBUF. Wt first (so ldweights never waits on it);
            # x^T in 4 chunks — one per queue semaphore, so each matmul's
            # single HW wait slot references exactly one DMA.
            Wt_s = small.tile([P, DC, HPC], f16, tag="Wt")
            nc.sync.dma_start(out=Wt_s, in_=Wt.rearrange("(c p) h -> p c h", p=P))
            b_s = small.tile([HPC, 1], f32, tag="b")
            nc.sync.dma_start(out=b_s, in_=bv[:])
            xT_s = big.tile([P, DC, N], f16, tag="xT")
            xT_r = xT.rearrange("(c p) n -> p c n", p=P)
            # last chunk kept small so the final matmul group retires right
            # after the input stream ends (per-c matmuls wait on whole DMAs)
            for lo, hi in ((0, 2), (2, 4), (4, 7), (7, 8)):
                nc.sync.dma_start(
                    out=xT_s[:, lo:hi, :], in_=xT_r[:, lo:hi, :]
                )
            nb = small.tile([HPC, 1], f32, tag="nb")
            nc.vector.tensor_scalar_mul(nb, b_s, -1.0)
            # one explicit load of the combined exp+ln+identity table, issued
            # while the x^T DMA streams; the table-load-insertion pass then
            # sees every activation below as covered and inserts nothing
            ACT_SET_LN_EXP = 6  # natural_log_exp_and_others in act_info.json
            nc.scalar.add_instruction(
                mybir.InstLoadActFuncSet(
                    name=f"I-{nc.next_id()}",
                    act_func_set_id=ACT_SET_LN_EXP,
                    engine=mybir.EngineType.Activation,
                )
            )

            ident = small.tile([HPC, HPC], f32, tag="ident")
            make_identity(nc, ident)
            zeros = small.tile([HPC, N], f32, tag="zeros")
            nc.gpsimd.memset(zeros, 0.0)

            u = small.tile([HPC, N], f32, tag="u")
            g = small.tile([HPC, N], f32, tag="g")
            ngcol = small.tile([P, NCH * HPC], f32, tag="ngcol")
            bcast = big.tile([P, HPC, N], f32, tag="bcast")

            # ---- logits^T [4, n]; each j-group accumulates over c in PSUM,
            # c-outer so group j can retire as soon as the last chunk lands
            # (moving free dim capped at 512 by the PSUM bank on the output)
            MV = 512
            ps = lps.tile([HPC, N], f32, tag="lps")
            for c in range(DC):
                for j in range(N // MV):
                    nc.tensor.matmul(
                        ps[:, j * MV : (j + 1) * MV],
                        Wt_s[:, c, :],
                        xT_s[:, c, j * MV : (j + 1) * MV],
                        start=(c == 0),
                        stop=(c == DC - 1),
                    )
            # t = exp(-(logits + b)); u = ln(1 + t)
            nc.scalar.activation(u, ps, Act.Exp, bias=nb[:, 0:1], scale=-1.0)
            nc.scalar.activation(u, u, Act.Ln, bias=1.0)
            # g = cumsum(u)
            nc.vector.tensor_tensor_scan(
                g, u, zeros, 0.0, mybir.AluOpType.add, mybir.AluOpType.add
            )

            # ---- negated g columns: ngcol[p, c*HPC + h] = -g[h, c*P + p]
            for c in range(NCH):
                gp = gps.tile([P, HPC], f32, tag="gps")
                nc.tensor.transpose(gp, g[:, c * P : (c + 1) * P], ident)
                nc.vector.tensor_scalar_mul(
                    ngcol[:, c * HPC : (c + 1) * HPC], gp, -1.0
                )

            gpscm.__exit__(None, None, None)
            ph1.__exit__(None, None, None)

            # ---- bcast[p, h, j] = g[h, j] via gpsimd partition_broadcast
            # (needs its source at partition 0: head 0 reads g directly,
            # heads 1-3 get their row moved down by a tiny SBUF->SBUF DMA)
            nc.gpsimd.partition_broadcast(bcast[:, 0, :], g[0:1, :])
            for h in range(1, HPC):
                grow = grp.tile([1, N], f32, tag="grow")
                nc.sync.dma_start(out=grow, in_=g[h : h + 1, :])
                nc.gpsimd.partition_broadcast(bcast[:, h, :], grow)

            # ---- out[h, c*P + p, :] = g[:] - g[h, c*P + p], fp16.
            # Alternate tiles between ScalarE (Identity+bias) and VectorE
            # (tensor_scalar_add, per-partition scalar) so tile generation
            # sustains ~2x one engine's rate and the DMA stream stays the
            # only roofline. (PSUM cannot be a DMA source, so tiles go via
            # SBUF either way.)
            for h in range(HPC):
                for c in range(NCH):
                    ot = outp.tile([P, N], f16, tag="ot")
                    col = c * HPC + h
                    if (h * NCH + c) % 2 == 0:
                        nc.scalar.activation(
                            ot,
                            bcast[:, h, :],
                            Act.Identity,
                            bias=ngcol[:, col : col + 1],
                            scale=1.0,
                        )
                    else:
                        nc.vector.tensor_scalar_add(
                            ot, bcast[:, h, :], ngcol[:, col : col + 1]
                        )
                    nc.sync.dma_start(out=out[h, c * P : (c + 1) * P, :], in_=ot)

    if not nc.is_finalized():
        nc.finalize()
    return nc


def _get_nc():
    if "nc" not in _CACHE:
        _CACHE["nc"] = _build_nc()
    return _CACHE["nc"]


def _make_in_maps(x, W, b):
    x = np.ascontiguousarray(x, dtype=np.float32)
    W = np.ascontiguousarray(W, dtype=np.float32)
    b = np.ascontiguousarray(b, dtype=np.float32)
    xT_by_batch = [np.ascontiguousarray(x[bi].T.astype(np.float16)) for bi in range(B)]
    in_maps = []
    for k in range(NCORES):
        bi = k // (NCORES // B)
        h0 = (k % (NCORES // B)) * HPC
        in_maps.append(
            {
                "xT": xT_by_batch[bi],
                "Wt": np.ascontiguousarray(W[h0 : h0 + HPC].T.astype(np.float16)),
                "bv": np.ascontiguousarray(b[h0 : h0 + HPC].reshape(HPC, 1)),
            }
        )
    return in_maps


def kernel(x, W, b, _trace=False, _trace_cores=None):
    from concourse.bass_utils import run_bass_kernel_spmd

    nc = _get_nc()
    in_maps = _make_in_maps(x, W, b)
    res = run_bass_kernel_spmd(
        nc, in_maps, core_ids=list(range(NCORES)), trace=_trace,
        trace_cores=_trace_cores,
    )
    _CACHE["last_results"] = res
    full = np.empty((B, NH, N, N), dtype=np.float32)
    for k in range(NCORES):
        bi = k // (NCORES // B)
        h0 = (k % (NCORES // B)) * HPC
        full[bi, h0 : h0 + HPC] = res.results[k]["out"]
    return full


# revision 10
# speedup vs baseline: 1.5838x; 1.1385x over previous
"""Data-dependent ALiBi bias kernel for Trainium2, distributed over 8 NeuronCores.

Reference computation (per full input):
    logits = einsum('bnd,hd->bhn', x, W) + b          # [2, 16, 2048]
    fg     = log_sigmoid(logits)                      # [2, 16, 2048]
    fg     = cumsum(fg, axis=-1)
    out    = fg[:, :, :, None] - fg[:, :, None, :]    # [2, 16, 2048, 2048]

Sharding: 32 (batch, head) pairs / 8 cores = 4 heads per core, batch-major
(cores 0-3 take batch 0, cores 4-7 take batch 1). Each core computes its own
[4, 2048, 2048] slab independently; no collectives.

Device algorithm per core (v2 — fp16 output stream):
    1. logits^T [4, n] via PE matmul of host-pre-transposed x^T (fp16) with
       W^T (fp16), fp32 PSUM accumulate; c-outer / j-inner so matmuls
       pipeline with the x^T chunk DMAs.
    2. u = ln(1 + exp(-(logits + b)))   (= -log_sigmoid(logits), via ACT
       Exp then Ln. An explicit early InstLoadActFuncSet of the combined
       natural_log_exp_and_others table — which also contains Identity —
       runs during the input-DMA wait and suppresses the framework's two
       per-function table loads on the critical path.)
    3. g = cumsum(u)                 (DVE tensor_tensor_scan; g = -fg_cum)
    4. out[h, i, j] = fg_cum[i] - fg_cum[j] = g[j] - g[i]:
       g rows replicated across all 128 partitions by gpsimd
       partition_broadcast (j-term); PE-transposed negated g columns give
       the per-partition i-term bias. Tiles are generated alternately by
       ScalarE (ACT Identity+bias) and VectorE (tensor_scalar_add with a
       per-partition scalar) so neither engine gates the fp16 DMA stream,
       and written as fp16 — host upcasts to fp32 on gather. fp16 rounding
       adds ~3e-4 Frobenius rel err (gate is 2e-2).

Output streaming is the roofline: 32 MB/core fp16 at the ~400 GB/s
sustained SBUF->DRAM DMA rate (~80 us) after a ~40 us lead-in.

Hardware gotchas baked into this design:
  - keep ACT Copy out of the ScalarE stream (table thrash on HW); all
    ScalarE ops here live in the one softplus table set.
  - PE matmul/transpose and partition_broadcast operands must sit at base
    partition 0 (or 32/64).
  - one HW wait slot per instruction: more input DMAs than queue
    semaphores gets waits consolidated into "wait for the last DMA".
"""

import numpy as np

B = 2
NH = 16
N = 2048
D = 1024
NCORES = 8
HPC = (B * NH) // NCORES  # 4 (batch, head) pairs per core
P = 128
DC = D // P    # 8 contraction chunks
NCH = N // P   # 16 row chunks per head

_CACHE = {}


def _build_nc():
    import concourse.bacc as bacc
    import concourse.mybir as mybir
    from concourse.masks import make_identity
    from concourse.tile import TileContext

    f32 = mybir.dt.float32
    f16 = mybir.dt.float16
    Act = mybir.ActivationFunctionType
    nc = bacc.Bacc(None, target_bir_lowering=False)

    # xT is host-pre-arranged partition-major: xT[p, c, n] = x^T[c*128+p, n],
    # so every input DMA descriptor is a multi-KB contiguous run per partition
    xT = nc.dram_tensor("xT", [P, DC, N], f16, kind="ExternalInput")
    Wt = nc.dram_tensor("Wt", [D, HPC], f16, kind="ExternalInput")
    bv = nc.dram_tensor("bv", [HPC, 1], f32, kind="ExternalInput")
    out = nc.dram_tensor("out", [HPC, N, N], f16, kind="ExternalOutput")

    with TileContext(nc) as tc:
        with (
            tc.tile_pool(name="big", bufs=1) as big,
            tc.tile_pool(name="small", bufs=1) as small,
            tc.tile_pool(name="grp", bufs=2) as grp,
            tc.tile_pool(name="outa", bufs=8) as outa,
            tc.tile_pool(name="outv", bufs=10) as outv,
        ):
            ph1 = tc.tile_pool(name="ph1ps", bufs=1, space="PSUM")
            lps = ph1.__enter__()
            gpscm = tc.tile_pool(name="gps", bufs=2, space="PSUM")
            gps = gpscm.__enter__()
            # ---- inputs -> SBUF. Wt first (so ldweights never waits on it);
            # x^T in 4 chunks — one per queue semaphore, so each matmul's
            # single HW wait slot references exactly one DMA.
            Wt_s = small.tile([P, DC, HPC], f16, tag="Wt")
            nc.sync.dma_start(out=Wt_s, in_=Wt.rearrange("(c p) h -> p c h", p=P))
            b_s = small.tile([HPC, 1], f32, tag="b")
            nc.sync.dma_start(out=b_s, in_=bv[:])
            xT_s = big.tile([P, DC, N], f16, tag="xT")
            # last chunk kept small so the final matmul group retires right
            # after the input stream ends (per-c matmuls wait on whole DMAs)
            for lo, hi in ((0, 2), (2, 4), (4, 7), (7, 8)):
                nc.sync.dma_start(
                    out=xT_s[:, lo:hi, :], in_=xT[:, lo:hi, :]
                )
            nb = small.tile([HPC, 1], f32, tag="nb")
            nc.vector.tensor_scalar_mul(nb, b_s, -1.0)
            # one explicit load of the combined exp+ln+identity table, issued
            # while the x^T DMA streams; the table-load-insertion pass then
            # sees every activation below as covered and inserts nothing
            ACT_SET_LN_EXP = 6  # natural_log_exp_and_others in act_info.json
            nc.scalar.add_instruction(
                mybir.InstLoadActFuncSet(
                    name=f"I-{nc.next_id()}",
                    act_func_set_id=ACT_SET_LN_EXP,
                    engine=mybir.EngineType.Activation,
                )
            )

            ident = small.tile([HPC, HPC], f32, tag="ident")
            make_identity(nc, ident)
            zeros = small.tile([HPC, N], f32, tag="zeros")
            nc.gpsimd.memset(zeros, 0.0)

            u = small.tile([HPC, N], f32, tag="u")
            g = small.tile([HPC, N], f32, tag="g")
            ngcol = small.tile([P, NCH * HPC], f32, tag="ngcol")
            bcast = big.tile([P, HPC, N], f32, tag="bcast")

            # ---- logits^T [4, n]; each j-group accumulates over c in PSUM,
            # c-outer so group j can retire as soon as the last chunk lands
            # (moving free dim capped at 512 by the PSUM bank on the output)
            MV = 512
            ps = lps.tile([HPC, N], f32, tag="lps")
            for c in range(DC):
                for j in range(N // MV):
                    nc.tensor.matmul(
                        ps[:, j * MV : (j + 1) * MV],
                        Wt_s[:, c, :],
                        xT_s[:, c, j * MV : (j + 1) * MV],
                        start=(c == 0),
                        stop=(c == DC - 1),
                    )
            # t = exp(-(logits + b)); u = ln(1 + t)
            nc.scalar.activation(u, ps, Act.Exp, bias=nb[:, 0:1], scale=-1.0)
            nc.scalar.activation(u, u, Act.Ln, bias=1.0)
            # g = cumsum(u)
            nc.vector.tensor_tensor_scan(
                g, u, zeros, 0.0, mybir.AluOpType.add, mybir.AluOpType.add
            )

            # ---- negated g columns: ngcol[p, c*HPC + h] = -g[h, c*P + p]
            for c in range(NCH):
                gp = gps.tile([P, HPC], f32, tag="gps")
                nc.tensor.transpose(gp, g[:, c * P : (c + 1) * P], ident)
                nc.vector.tensor_scalar_mul(
                    ngcol[:, c * HPC : (c + 1) * HPC], gp, -1.0
                )

            gpscm.__exit__(None, None, None)
            ph1.__exit__(None, None, None)

            # ---- bcast[p, h, j] = g[h, j] via gpsimd partition_broadcast
            # (needs its source at partition 0: head 0 reads g directly,
            # heads 1-3 get their row moved down by a tiny SBUF->SBUF DMA)
            nc.gpsimd.partition_broadcast(bcast[:, 0, :], g[0:1, :])
            for h in range(1, HPC):
                grow = grp.tile([1, N], f32, tag="grow")
                nc.sync.dma_start(out=grow, in_=g[h : h + 1, :])
                nc.gpsimd.partition_broadcast(bcast[:, h, :], grow)

            # ---- out[h, c*P + p, :] = g[:] - g[h, c*P + p], fp16.
            # Alternate tiles between ScalarE (Identity+bias) and VectorE
            # (tensor_scalar_add, per-partition scalar) so tile generation
            # sustains ~2x one engine's rate and the DMA stream stays the
            # only roofline. (PSUM cannot be a DMA source, so tiles go via
            # SBUF either way.)
            # measured rates: ACT ~2.35 us/tile, DVE ~1.54 us/tile -> 3:5 split
            for h in range(HPC):
                for c in range(NCH):
                    col = c * HPC + h
                    idx = h * NCH + c
                    if idx % 8 in (0, 3, 5):
                        ot = outa.tile([P, N], f16, tag="ota")
                        nc.scalar.activation(
                            ot,
                            bcast[:, h, :],
                            Act.Identity,
                            bias=ngcol[:, col : col + 1],
                            scale=1.0,
                        )
                    else:
                        ot = outv.tile([P, N], f16, tag="otv")
                        nc.vector.tensor_scalar_add(
                            ot, bcast[:, h, :], ngcol[:, col : col + 1]
                        )
                    nc.sync.dma_start(out=out[h, c * P : (c + 1) * P, :], in_=ot)

    if not nc.is_finalized():
        nc.finalize()
    return nc


def _get_nc():
    if "nc" not in _CACHE:
        _CACHE["nc"] = _build_nc()
    return _CACHE["nc"]


def _make_in_maps(x, W, b):
    x = np.ascontiguousarray(x, dtype=np.float32)
    W = np.ascontiguousarray(W, dtype=np.float32)
    b = np.ascontiguousarray(b, dtype=np.float32)
    # partition-major layout: xT[p, c, n] = x[bi].T[c*128+p, n]
    xT_by_batch = [
        np.ascontiguousarray(
            x[bi].T.astype(np.float16).reshape(DC, P, N).transpose(1, 0, 2)
        )
        for bi in range(B)
    ]
    in_maps = []
    for k in range(NCORES):
        bi = k // (NCORES // B)
        h0 = (k % (NCORES // B)) * HPC
        in_maps.append(
            {
                "xT": xT_by_batch[bi],
                "Wt": np.ascontiguousarray(W[h0 : h0 + HPC].T.astype(np.float16)),
                "bv": np.ascontiguousarray(b[h0 : h0 + HPC].reshape(HPC, 1)),
            }
        )
    return in_maps


def kernel(x, W, b, _trace=False, _trace_cores=None):
    from concourse.bass_utils import run_bass_kernel_spmd

    nc = _get_nc()
    in_maps = _make_in_maps(x, W, b)
    res = run_bass_kernel_spmd(
        nc, in_maps, core_ids=list(range(NCORES)), trace=_trace,
        trace_cores=_trace_cores,
    )
    _CACHE["last_results"] = res
    full = np.empty((B, NH, N, N), dtype=np.float32)
    for k in range(NCORES):
        bi = k // (NCORES // B)
        h0 = (k % (NCORES // B)) * HPC
        full[bi, h0 : h0 + HPC] = res.results[k]["out"]
    return full
